# revision 11
# baseline (speedup 1.0000x reference)
"""Trainium2 Bass kernel for ChainRelativePositionEmbedding.

Problem: out[0, i, j, :] = Wt[1 + ridx_finl(i,j)] + same_chain(i,j) * Wt[0] + bias
with 3 chains of 512 residues (L = 1536), Wt = weight.T [67, 128].

Every output pair-vector is one of only 66 distinct 128-float vectors:
  same chain:  T_same[k] = Wt[1+k] + Wt[0] + bias,  k = clip(p_i - p_j + 32, 0, 64)
  cross chain: T_diff    = Wt[66] + bias

So the kernel is pure DMA replication out of a tiny SBUF-resident table — no
compute engines at all. Work is sharded across 8 cores with an INTERLEAVED row
assignment (core c owns global rows i == c (mod 8)), which makes the Bass
program identical on every core:
  * local row r in [0,192): chain b = r//64, r' = r%64, residue p = 8*r' + c.
  * diagonal (same-chain) block of each row is a 512-row sliding window into a
    1024-entry "master" strip laid out [128 partitions x 8 vectors]; with the
    stride-8 row interleave the window start 511 - 8*r' is always ≡ 7 (mod 8),
    so every window is a single rectangular SBUF access pattern
    [partitions 63-r' .. 127-r') x [full 4 KiB free dim].
  * the core index c only shifts the CONTENT of the master strip, which is a
    per-core input built on the host from weight/bias.
  * the two cross-chain blocks of each row are T_diff replicated; they are
    written from a 4 MiB SBUF constant region, 8 output rows per dma_start.
Per core: 2 input DMAs (4.5 MiB) + 224 output DMAs (151 MiB written).
"""

import numpy as np

import concourse.bass as bass
import concourse.mybir as mybir
from concourse.bass_utils import run_bass_kernel_spmd

L = 1536          # total residues (3 chains x 512)
D = 128           # embedding dim
NCORES = 8
RPC = L // NCORES  # rows per core = 192

# Module-level knobs/results (used by test.py; harness just calls kernel()).
TRACE = False
TRACE_KWARGS = {}
LAST_RESULTS = None

_CACHED_NC = None


def _build_nc():
    nc = bass.Bass()
    f32 = mybir.dt.float32

    master = nc.declare_dram_parameter("master", [128, 1024], f32, isOutput=False)
    constsrc = nc.declare_dram_parameter("constsrc", [128, 1024], f32, isOutput=False)
    out = nc.declare_dram_parameter("out", [RPC, L, D], f32, isOutput=True)

    # View with the three 512-col j-blocks split out, so a chain-1 row's two
    # cross-chain blocks {0, 2} are a single strided AP.
    out_b = out.rearrange("r (b j) d -> r b (j d)", b=3)

    with (
        nc.sbuf_tensor("msb", [128, 1024], f32) as msb,
        nc.sbuf_tensor("csb", [128, 1024], f32) as csb,
        nc.semaphore("dsem") as dsem,
        nc.Block() as block,
    ):
        # ---- output DMA job lists (dst AP, src AP) ----
        # All jobs balance to <=3-dim APs with 1024-element (4 KiB) final
        # dims: 4 KiB descriptors spread across all 16 SDMA engines (bigger
        # per-partition runs were observed to land on only 8 engines).
        #
        # Const traffic (96 MiB of identical T_diff vectors) is 4 giant DMAs
        # using a step-0 (broadcast) middle dim on the SBUF source — one
        # continuous descriptor stream with no per-DMA completion stalls.
        # Every csb partition holds identical content, so the partition-major
        # element order of the broadcast read matches any dst order at 512 B
        # granularity.
        def cbc(reps):
            return csb[:, :].unsqueeze(1).broadcast_to([128, reps, 1024])

        const_jobs = [
            (out[0:64, 512:1536, :], cbc(64)),     # chain 0 rows: j in [512,1536)
            (out[64:128, 0:512, :], cbc(32)),      # chain 1 rows: j in [0,512)
            (out[64:128, 1024:1536, :], cbc(32)),  # chain 1 rows: j in [1024,1536)
            (out[128:192, 0:1024, :], cbc(64)),    # chain 2 rows: j in [0,1024)
        ]
        diag_jobs = []
        for r in range(RPC):
            b, rp = r // 64, r % 64
            # diag: 512 consecutive master entries starting at partition
            # 63-rp, read partition-major -> the same-chain block of row r.
            diag_jobs.append(
                (out[r, 512 * b : 512 * (b + 1), :], msb[63 - rp : 127 - rp, :])
            )
        total_incs = 16 * (2 + len(const_jobs) + len(diag_jobs))

        # The ring-serial cost is ~2.5 us per dma_start (trailing-semaphore
        # write receipt), so a single ring canNOT retire 192 diag DMAs in
        # less than ~480 us regardless of size. Split the diag chain across
        # THREE descriptor-generation rings (sync HWDGE, scalar HWDGE,
        # gpsimd SWDGE) so the stall chains run in parallel, and stage the
        # big const streams so one ring is always streaming densely.
        diag3 = [diag_jobs[0::3], diag_jobs[1::3], diag_jobs[2::3]]

        @block.sync
        def _(eng):
            eng.dma_start(out=msb[:, :], in_=master[:, :]).then_inc(dsem, 16)
            eng.dma_start(out=csb[:, :], in_=constsrc[:, :]).then_inc(dsem, 16)
            eng.wait_ge(dsem, 32)
            for dst, src in diag3[0]:
                eng.dma_start(out=dst, in_=src).then_inc(dsem, 16)
            for dst, src in const_jobs[0:2]:
                eng.dma_start(out=dst, in_=src).then_inc(dsem, 16)
            eng.wait_ge(dsem, total_incs)

        @block.scalar
        def _(eng):
            eng.wait_ge(dsem, 32)
            for dst, src in const_jobs[2:4]:
                eng.dma_start(out=dst, in_=src).then_inc(dsem, 16)
            for dst, src in diag3[1]:
                eng.dma_start(out=dst, in_=src).then_inc(dsem, 16)

        @block.gpsimd
        def _(eng):
            eng.wait_ge(dsem, 32)
            for dst, src in diag3[2]:
                eng.dma_start(out=dst, in_=src).then_inc(dsem, 16)

    return nc


def _expected_asym_id():
    return np.repeat(np.arange(1, 4, dtype=np.int32), 512)


def _fallback_numpy(lengths, asym_id, weight, bias):
    """Generic host path if inputs ever deviate from the hardcoded structure."""
    lengths = np.asarray(lengths).astype(np.int64)
    asym_id = np.asarray(asym_id)
    weight = np.asarray(weight, np.float32)
    bias = np.asarray(bias, np.float32)
    ridx_max = (weight.shape[1] - 3) // 2
    idxs = np.concatenate([np.arange(int(l), dtype=np.int32) for l in lengths])
    asym_mat = asym_id[:, None] == asym_id[None, :]
    ridx = idxs[:, None] - idxs[None, :]
    ridx_clip = np.clip(ridx + ridx_max, 0, 2 * ridx_max)
    ridx_finl = np.where(asym_mat, ridx_clip, 2 * ridx_max + 1)
    Wt = weight.T
    pfea = Wt[1 + ridx_finl] + asym_mat.astype(weight.dtype)[..., None] * Wt[0] + bias
    return pfea[None]


def kernel(lengths=None, asym_id=None, weight=None, bias=None):
    global _CACHED_NC, LAST_RESULTS

    lengths = np.asarray(lengths)
    asym_id = np.asarray(asym_id)
    weight = np.asarray(weight, np.float32)
    bias = np.asarray(bias, np.float32)

    if (
        weight.shape != (D, 67)
        or tuple(lengths.astype(np.int64)) != (512, 512, 512)
        or asym_id.shape != (L,)
        or not np.array_equal(asym_id, _expected_asym_id())
    ):
        return _fallback_numpy(lengths, asym_id, weight, bias)

    # Combined lookup tables (same float op order as the reference).
    Wt = weight.T                           # [67, 128]
    T_same = Wt[1:66] + Wt[0] + bias        # [65, 128]
    T_diff = (Wt[66] + bias).astype(np.float32)  # [128]

    # Per-core master strip: master_c[u] = T_same[clip(543 + c - u, 0, 64)],
    # laid out [partition p, vector f] with u = 7 + 8p + f.
    u = 7 + 8 * np.arange(128)[:, None] + np.arange(8)[None, :]  # [128, 8]
    const_np = np.ascontiguousarray(np.tile(T_diff, (128, 8)))  # [128, 1024]

    in_maps = []
    for c in range(NCORES):
        idx = np.clip(543 + c - u, 0, 64)
        master_np = np.ascontiguousarray(T_same[idx].reshape(128, 1024))
        in_maps.append({"master": master_np, "constsrc": const_np})

    if _CACHED_NC is None:
        _CACHED_NC = _build_nc()

    res = run_bass_kernel_spmd(
        _CACHED_NC,
        in_maps,
        list(range(NCORES)),
        trace=TRACE,
        **TRACE_KWARGS,
    )
    LAST_RESULTS = res

    full = np.empty((L, L, D), np.float32)
    for c in range(NCORES):
        full[c::8] = res.results[c]["out"]
    return full[None]


# revision 14
# speedup vs baseline: 1.0178x; 1.0178x over previous
"""Trainium2 Bass kernel for ChainRelativePositionEmbedding.

Problem: out[0, i, j, :] = Wt[1 + ridx_finl(i,j)] + same_chain(i,j) * Wt[0] + bias
with 3 chains of 512 residues (L = 1536), Wt = weight.T [67, 128].

Every output pair-vector is one of only 66 distinct 128-float vectors:
  same chain:  T_same[k] = Wt[1+k] + Wt[0] + bias,  k = clip(p_i - p_j + 32, 0, 64)
  cross chain: T_diff    = Wt[66] + bias

So the kernel is pure DMA replication out of a tiny SBUF-resident table — no
compute engines at all. Work is sharded across 8 cores with an INTERLEAVED row
assignment (core c owns global rows i == c (mod 8)), which makes the Bass
program identical on every core:
  * local row r in [0,192): chain b = r//64, r' = r%64, residue p = 8*r' + c.
  * diagonal (same-chain) block of each row is a 512-row sliding window into a
    1024-entry "master" strip laid out [128 partitions x 8 vectors]; with the
    stride-8 row interleave the window start 511 - 8*r' is always ≡ 7 (mod 8),
    so every window is a single rectangular SBUF access pattern
    [partitions 63-r' .. 127-r') x [full 4 KiB free dim].
  * the core index c only shifts the CONTENT of the master strip, which is a
    per-core input built on the host from weight/bias.
  * the two cross-chain blocks of each row are T_diff replicated; they are
    written from a 4 MiB SBUF constant region, 8 output rows per dma_start.
Per core: 2 input DMAs (4.5 MiB) + 224 output DMAs (151 MiB written).
"""

import numpy as np

import concourse.bass as bass
import concourse.mybir as mybir
from concourse.bass_utils import run_bass_kernel_spmd

L = 1536          # total residues (3 chains x 512)
D = 128           # embedding dim
NCORES = 8
RPC = L // NCORES  # rows per core = 192

# Module-level knobs/results (used by test.py; harness just calls kernel()).
TRACE = False
TRACE_KWARGS = {}
LAST_RESULTS = None

_CACHED_NC = None


def _build_nc():
    nc = bass.Bass()
    f32 = mybir.dt.float32

    master = nc.declare_dram_parameter("master", [128, 1024], f32, isOutput=False)
    constsrc = nc.declare_dram_parameter("constsrc", [128, 1024], f32, isOutput=False)
    out = nc.declare_dram_parameter("out", [RPC, L, D], f32, isOutput=True)

    # View with the three 512-col j-blocks split out, so a chain-1 row's two
    # cross-chain blocks {0, 2} are a single strided AP.
    out_b = out.rearrange("r (b j) d -> r b (j d)", b=3)

    with (
        nc.sbuf_tensor("msb", [128, 1024], f32) as msb,
        nc.sbuf_tensor("csb", [128, 1024], f32) as csb,
        nc.semaphore("dsem") as dsem,
        nc.Block() as block,
    ):
        # ---- output DMA job lists (dst AP, src AP) ----
        # All jobs balance to <=3-dim APs with 1024-element (4 KiB) final
        # dims: 4 KiB descriptors spread across all 16 SDMA engines (bigger
        # per-partition runs were observed to land on only 8 engines).
        #
        # Const traffic (96 MiB of identical T_diff vectors) is 4 giant DMAs
        # using a step-0 (broadcast) middle dim on the SBUF source — one
        # continuous descriptor stream with no per-DMA completion stalls.
        # Every csb partition holds identical content, so the partition-major
        # element order of the broadcast read matches any dst order at 512 B
        # granularity.
        def cbc(reps):
            return csb[:, :].unsqueeze(1).broadcast_to([128, reps, 1024])

        const_jobs = [
            (out[0:64, 512:1536, :], cbc(64)),     # chain 0 rows: j in [512,1536)
            (out[64:128, 0:512, :], cbc(32)),      # chain 1 rows: j in [0,512)
            (out[64:128, 1024:1536, :], cbc(32)),  # chain 1 rows: j in [1024,1536)
            (out[128:192, 0:1024, :], cbc(64)),    # chain 2 rows: j in [0,1024)
        ]
        diag_jobs = []
        for r in range(RPC):
            b, rp = r // 64, r % 64
            # diag: 512 consecutive master entries starting at partition
            # 63-rp, read partition-major -> the same-chain block of row r.
            diag_jobs.append(
                (out[r, 512 * b : 512 * (b + 1), :], msb[63 - rp : 127 - rp, :])
            )
        total_incs = 16 * (2 + len(const_jobs) + len(diag_jobs))

        @block.sync
        def _(eng):
            eng.dma_start(out=msb[:, :], in_=master[:, :]).then_inc(dsem, 16)
            eng.dma_start(out=csb[:, :], in_=constsrc[:, :]).then_inc(dsem, 16)
            eng.wait_ge(dsem, 32)
            for dst, src in diag_jobs:
                eng.dma_start(out=dst, in_=src).then_inc(dsem, 16)
            eng.wait_ge(dsem, total_incs)

        @block.scalar
        def _(eng):
            eng.wait_ge(dsem, 32)
            for dst, src in const_jobs:
                eng.dma_start(out=dst, in_=src).then_inc(dsem, 16)

    return nc


def _expected_asym_id():
    return np.repeat(np.arange(1, 4, dtype=np.int32), 512)


def _fallback_numpy(lengths, asym_id, weight, bias):
    """Generic host path if inputs ever deviate from the hardcoded structure."""
    lengths = np.asarray(lengths).astype(np.int64)
    asym_id = np.asarray(asym_id)
    weight = np.asarray(weight, np.float32)
    bias = np.asarray(bias, np.float32)
    ridx_max = (weight.shape[1] - 3) // 2
    idxs = np.concatenate([np.arange(int(l), dtype=np.int32) for l in lengths])
    asym_mat = asym_id[:, None] == asym_id[None, :]
    ridx = idxs[:, None] - idxs[None, :]
    ridx_clip = np.clip(ridx + ridx_max, 0, 2 * ridx_max)
    ridx_finl = np.where(asym_mat, ridx_clip, 2 * ridx_max + 1)
    Wt = weight.T
    pfea = Wt[1 + ridx_finl] + asym_mat.astype(weight.dtype)[..., None] * Wt[0] + bias
    return pfea[None]


def kernel(lengths=None, asym_id=None, weight=None, bias=None):
    global _CACHED_NC, LAST_RESULTS

    lengths = np.asarray(lengths)
    asym_id = np.asarray(asym_id)
    weight = np.asarray(weight, np.float32)
    bias = np.asarray(bias, np.float32)

    if (
        weight.shape != (D, 67)
        or tuple(lengths.astype(np.int64)) != (512, 512, 512)
        or asym_id.shape != (L,)
        or not np.array_equal(asym_id, _expected_asym_id())
    ):
        return _fallback_numpy(lengths, asym_id, weight, bias)

    # Combined lookup tables (same float op order as the reference).
    Wt = weight.T                           # [67, 128]
    T_same = Wt[1:66] + Wt[0] + bias        # [65, 128]
    T_diff = (Wt[66] + bias).astype(np.float32)  # [128]

    # Per-core master strip: master_c[u] = T_same[clip(543 + c - u, 0, 64)],
    # laid out [partition p, vector f] with u = 7 + 8p + f.
    u = 7 + 8 * np.arange(128)[:, None] + np.arange(8)[None, :]  # [128, 8]
    const_np = np.ascontiguousarray(np.tile(T_diff, (128, 8)))  # [128, 1024]

    in_maps = []
    for c in range(NCORES):
        idx = np.clip(543 + c - u, 0, 64)
        master_np = np.ascontiguousarray(T_same[idx].reshape(128, 1024))
        in_maps.append({"master": master_np, "constsrc": const_np})

    if _CACHED_NC is None:
        _CACHED_NC = _build_nc()

    res = run_bass_kernel_spmd(
        _CACHED_NC,
        in_maps,
        list(range(NCORES)),
        trace=TRACE,
        **TRACE_KWARGS,
    )
    LAST_RESULTS = res

    full = np.empty((L, L, D), np.float32)
    for c in range(NCORES):
        full[c::8] = res.results[c]["out"]
    return full[None]
